# revision 8
# baseline (speedup 1.0000x reference)
"""Trainium2 Bass kernel for single-head attention (B=8, S=2048, D=U=512).

Sharding: data-parallel over batch — one batch element per NeuronCore (8 cores).

Per-core dataflow (all matmuls in float32r):
  1. PE-transpose query/value into XT/VT ([d on partitions, s free]).
  2. Projections: qT = W1^T X^T, kT = W2^T V^T  ([u part, s free]);
     v = V W3 natural ([s part, u free]).
  3. Attention over i-blocks of 512 query positions:
     scoresT[j, i] = sum_u kT[u,j] qT[u,i]   (PSUM, accumulated over u-chunks)
     expS = exp(scoresT / sqrt(U))           (ScalarE, written f32r to SBUF)
     ctx[i, u]  = sum_j expS[j,i] v[j,u]     (PE, expS chunks stationary)
     den[i]     = sum_j expS[j,i]            (PE, ones vector moving)
     out[i, :]  = ctx[i, :] / den[i]         (DVE reciprocal + tensor_scalar)
  Softmax max-subtraction is skipped: scores ~ N(0,1), exp() cannot overflow.
"""

import math
import os
import sys

for _p in ("/opt/trn_rl_repo", os.path.expanduser("~/.axon_site/_ro/trn_rl_repo")):
    if os.path.isdir(_p) and _p not in sys.path:
        sys.path.insert(0, _p)

import numpy as np

import concourse.bass as bass
import concourse.tile as tile
from concourse import bacc, mybir
from concourse.bass import ts
from concourse.bass_utils import run_bass_kernel_spmd
from concourse.masks import make_identity

F32 = mybir.dt.float32
F32R = mybir.dt.float32r
EXP = mybir.ActivationFunctionType.Exp

P = 128          # partitions
B = 8            # batch (one element per core)
S = 2048         # sequence length
D = 512          # model dim
U = 512          # units
DC = D // P      # 4 d-chunks
UC = U // P      # 4 u-chunks
SC = S // P      # 16 s-chunks
IB = 512         # i-block (query positions per attention block)
NIB = S // IB    # 4
ICC = IB // P    # 4 i-chunks per block
SCALE = 1.0 / math.sqrt(float(U))


def _emit(nc, tc, q_d, v_d, w1_d, w2_d, w3_d, o_d):
    with tc.tile_pool(name="const", bufs=1) as cp:
        ident = cp.tile([P, P], F32, name="ident")
        make_identity(nc, ident)
        # fp32r matmul needs even free sizes, so the all-ones moving operand
        # (and the denominator PSUM tile) are 2 columns wide.
        ones32 = cp.tile([P, 2], F32, name="ones32")
        nc.vector.memset(ones32, 1.0)
        ones = cp.tile([P, 2], F32R, name="ones")
        nc.vector.tensor_copy(ones, ones32)

        with tc.tile_pool(name="wpool", bufs=1) as wp:
            w1 = wp.tile([P, DC, U], F32R, name="w1")
            w2 = wp.tile([P, DC, U], F32R, name="w2")
            w3 = wp.tile([P, DC, U], F32R, name="w3")

            with tc.tile_pool(name="qkv", bufs=1) as qkvp:
                qT = qkvp.tile([P, UC, S], F32R, name="qT")
                kT = qkvp.tile([P, UC, S], F32R, name="kT")
                vN = qkvp.tile([P, SC, U], F32R, name="vN")

                # ---- phase 1: transposes + projections ----
                with tc.tile_pool(name="xtp", bufs=1) as xtp, \
                     tc.tile_pool(name="loadp", bufs=4) as loadp, \
                     tc.tile_pool(name="tps", bufs=3, space="PSUM") as tpsp, \
                     tc.tile_pool(name="pjps", bufs=3, space="PSUM") as pjps:
                    xT = xtp.tile([P, DC, S], F32R, name="xT")
                    vT = xtp.tile([P, DC, S], F32R, name="vT")

                    # PSUM->SBUF copies alternate between DVE and ACT so
                    # neither engine gates the PE transpose/matmul stream.
                    _cp_idx = [0]

                    def copy_out(dst, src):
                        _cp_idx[0] += 1
                        if _cp_idx[0] % 2:
                            nc.vector.tensor_copy(dst, src)
                        else:
                            nc.scalar.copy(dst, src)

                    # V/X natural loads first (transposes gate everything);
                    # weight DMAs are emitted after the first two loads so
                    # they don't delay the first transpose.
                    nats = {}
                    for i, (src_d, key) in enumerate(((v_d, "v"), (q_d, "x"))):
                        for sc in range(SC):
                            nat = loadp.tile([P, D], F32R, tag="nat",
                                             name=f"nat_{key}{sc}")
                            nc.sync.dma_start(nat, src_d[ts(sc, P), :])
                            nats[(key, sc)] = nat
                            if i == 0 and sc == 1:
                                nc.sync.dma_start(
                                    w2, w2_d.rearrange("(c p) u -> p c u", p=P))
                                nc.sync.dma_start(
                                    w3, w3_d.rearrange("(c p) u -> p c u", p=P))
                                nc.sync.dma_start(
                                    w1, w1_d.rearrange("(c p) u -> p c u", p=P))
                            # 4 transposes share one PSUM bank; single copy out
                            dstT = vT if key == "v" else xT
                            tp = tpsp.tile([P, DC * P], F32, tag="tp")
                            for dc in range(DC):
                                nc.tensor.transpose(
                                    tp[:, ts(dc, P)], nat[:, ts(dc, P)].bitcast(F32),
                                    ident)
                            copy_out(dstT[:, :, ts(sc, P)],
                                     tp.rearrange("p (c q) -> p c q", c=DC))

                    # kT = W2^T V^T
                    for uc in range(UC):
                        for ib in range(NIB):
                            ps = pjps.tile([P, IB], F32, tag="pj")
                            for dc in range(DC):
                                nc.tensor.matmul(
                                    ps, w2[:, dc, ts(uc, P)], vT[:, dc, ts(ib, IB)],
                                    start=(dc == 0), stop=(dc == DC - 1))
                            copy_out(kT[:, uc, ts(ib, IB)], ps)
                    # v natural = V W3
                    for jc in range(SC):
                        ps = pjps.tile([P, U], F32, tag="pj")
                        for dc in range(DC):
                            nc.tensor.matmul(
                                ps, vT[:, dc, ts(jc, P)], w3[:, dc, :],
                                start=(dc == 0), stop=(dc == DC - 1))
                        copy_out(vN[:, jc, :], ps)
                    # qT = W1^T X^T  (i-block outer so block 0 finishes first
                    # and attention can begin)
                    for ib in range(NIB):
                        for uc in range(UC):
                            ps = pjps.tile([P, IB], F32, tag="pj")
                            for dc in range(DC):
                                nc.tensor.matmul(
                                    ps, w1[:, dc, ts(uc, P)], xT[:, dc, ts(ib, IB)],
                                    start=(dc == 0), stop=(dc == DC - 1))
                            copy_out(qT[:, uc, ts(ib, IB)], ps)

                # ---- phase 2: attention ----
                with tc.tile_pool(name="expp", bufs=2) as expp, \
                     tc.tile_pool(name="scps", bufs=3, space="PSUM") as scps, \
                     tc.tile_pool(name="ctps", bufs=2, space="PSUM") as ctps, \
                     tc.tile_pool(name="dnps", bufs=2, space="PSUM") as dnps, \
                     tc.tile_pool(name="outp", bufs=3) as outp:
                    for ib in range(NIB):
                        expB = expp.tile([P, SC, IB], F32R, name="expB")
                        for jc in range(SC):
                            ps = scps.tile([P, IB], F32, tag="sc")
                            for uc in range(UC):
                                nc.tensor.matmul(
                                    ps, kT[:, uc, ts(jc, P)], qT[:, uc, ts(ib, IB)],
                                    start=(uc == 0), stop=(uc == UC - 1))
                            nc.scalar.activation(expB[:, jc, :], ps, EXP, scale=SCALE)
                        for icc in range(ICC):
                            i_glob = ib * ICC + icc
                            dps = dnps.tile([P, 2], F32, tag="dn")
                            for jc in range(SC):
                                nc.tensor.matmul(
                                    dps, expB[:, jc, ts(icc, P)], ones,
                                    start=(jc == 0), stop=(jc == SC - 1))
                            recip = outp.tile([P, 1], F32, tag="recip")
                            nc.vector.reciprocal(recip, dps[:, 0:1])
                            cps = ctps.tile([P, U], F32, tag="ct")
                            for jc in range(SC):
                                nc.tensor.matmul(
                                    cps, expB[:, jc, ts(icc, P)], vN[:, jc, :],
                                    start=(jc == 0), stop=(jc == SC - 1))
                            co = outp.tile([P, U], F32, tag="co")
                            nc.vector.tensor_scalar_mul(co, cps, recip)
                            nc.sync.dma_start(o_d[ts(i_glob, P), :], co)


_PROGRAM = None


def _get_program():
    global _PROGRAM
    if _PROGRAM is None:
        nc = bacc.Bacc("TRN2", target_bir_lowering=False, debug=False,
                       num_devices=B)
        q_d = nc.dram_tensor("query", (S, D), F32R, kind="ExternalInput").ap()
        v_d = nc.dram_tensor("value", (S, D), F32R, kind="ExternalInput").ap()
        w1_d = nc.dram_tensor("W1", (D, U), F32R, kind="ExternalInput").ap()
        w2_d = nc.dram_tensor("W2", (D, U), F32R, kind="ExternalInput").ap()
        w3_d = nc.dram_tensor("W3", (D, U), F32R, kind="ExternalInput").ap()
        o_d = nc.dram_tensor("out", (S, U), F32, kind="ExternalOutput").ap()
        with tile.TileContext(nc) as tc:
            _emit(nc, tc, q_d, v_d, w1_d, w2_d, w3_d, o_d)
        nc.compile()
        _PROGRAM = nc
    return _PROGRAM


def kernel(**inputs) -> np.ndarray:
    query = np.ascontiguousarray(inputs["query"], dtype=np.float32)
    value = np.ascontiguousarray(inputs["value"], dtype=np.float32)
    W1 = np.ascontiguousarray(inputs["W1"], dtype=np.float32)
    W2 = np.ascontiguousarray(inputs["W2"], dtype=np.float32)
    W3 = np.ascontiguousarray(inputs["W3"], dtype=np.float32)
    assert query.shape == (B, S, D) and value.shape == (B, S, D)

    nc = _get_program()
    in_maps = [
        {"query": query[b], "value": value[b], "W1": W1, "W2": W2, "W3": W3}
        for b in range(B)
    ]
    res = run_bass_kernel_spmd(nc, in_maps, core_ids=list(range(B)))
    return np.stack([res.results[b]["out"] for b in range(B)], axis=0)


# revision 10
# speedup vs baseline: 1.0005x; 1.0005x over previous
"""Trainium2 Bass kernel for single-head attention (B=8, S=2048, D=U=512).

Sharding: data-parallel over batch — one batch element per NeuronCore (8 cores).

Per-core dataflow (all matmuls in float32r):
  1. PE-transpose query/value into XT/VT ([d on partitions, s free]).
  2. Projections: qT = W1^T X^T, kT = W2^T V^T  ([u part, s free]);
     v = V W3 natural ([s part, u free]).
  3. Attention over i-blocks of 512 query positions:
     scoresT[j, i] = sum_u kT[u,j] qT[u,i]   (PSUM, accumulated over u-chunks)
     expS = exp(scoresT / sqrt(U))           (ScalarE, written f32r to SBUF)
     ctx[i, u]  = sum_j expS[j,i] v[j,u]     (PE, expS chunks stationary)
     den[i]     = sum_j expS[j,i]            (PE, ones vector moving)
     out[i, :]  = ctx[i, :] / den[i]         (DVE reciprocal + tensor_scalar)
  Softmax max-subtraction is skipped: scores ~ N(0,1), exp() cannot overflow.
"""

import math
import os
import sys

for _p in ("/opt/trn_rl_repo", os.path.expanduser("~/.axon_site/_ro/trn_rl_repo")):
    if os.path.isdir(_p) and _p not in sys.path:
        sys.path.insert(0, _p)

import numpy as np

import concourse.bass as bass
import concourse.tile as tile
from concourse import bacc, mybir
from concourse.bass import ts
from concourse.bass_utils import run_bass_kernel_spmd
from concourse.masks import make_identity

F32 = mybir.dt.float32
F32R = mybir.dt.float32r
EXP = mybir.ActivationFunctionType.Exp

P = 128          # partitions
B = 8            # batch (one element per core)
S = 2048         # sequence length
D = 512          # model dim
U = 512          # units
DC = D // P      # 4 d-chunks
UC = U // P      # 4 u-chunks
SC = S // P      # 16 s-chunks
IB = 512         # i-block (query positions per attention block)
NIB = S // IB    # 4
ICC = IB // P    # 4 i-chunks per block
SCALE = 1.0 / math.sqrt(float(U))


def _emit(nc, tc, q_d, v_d, w1_d, w2_d, w3_d, o_d):
    with tc.tile_pool(name="const", bufs=1) as cp:
        ident = cp.tile([P, P], F32, name="ident")
        make_identity(nc, ident)
        # Stationary all-ones operand for softmax denominators: [K=128, M=2]
        # (fp32r needs even sizes; only output row 0 is used).
        ones32 = cp.tile([P, 2], F32, name="ones32")
        nc.vector.memset(ones32, 1.0)
        ones = cp.tile([P, 2], F32R, name="ones")
        nc.vector.tensor_copy(ones, ones32)

        with tc.tile_pool(name="wpool", bufs=1) as wp:
            w1 = wp.tile([P, DC, U], F32R, name="w1")
            w2 = wp.tile([P, DC, U], F32R, name="w2")
            w3 = wp.tile([P, DC, U], F32R, name="w3")

            with tc.tile_pool(name="qkv", bufs=1) as qkvp:
                qT = qkvp.tile([P, UC, S], F32R, name="qT")
                kT = qkvp.tile([P, UC, S], F32R, name="kT")
                vN = qkvp.tile([P, SC, U], F32R, name="vN")

                # ---- phase 1: transposes + projections ----
                with tc.tile_pool(name="xtp", bufs=1) as xtp, \
                     tc.tile_pool(name="loadp", bufs=4) as loadp, \
                     tc.tile_pool(name="tps", bufs=3, space="PSUM") as tpsp, \
                     tc.tile_pool(name="pjps", bufs=3, space="PSUM") as pjps:
                    xT = xtp.tile([P, DC, S], F32R, name="xT")
                    vT = xtp.tile([P, DC, S], F32R, name="vT")

                    # PSUM->SBUF copies alternate between DVE and ACT so
                    # neither engine gates the PE transpose/matmul stream.
                    _cp_idx = [0]

                    def copy_out(dst, src):
                        _cp_idx[0] += 1
                        if _cp_idx[0] % 2:
                            nc.vector.tensor_copy(dst, src)
                        else:
                            nc.scalar.copy(dst, src)

                    # V/X natural loads first (transposes gate everything);
                    # weight DMAs are emitted after the first two loads so
                    # they don't delay the first transpose.
                    nats = {}
                    for i, (src_d, key) in enumerate(((v_d, "v"), (q_d, "x"))):
                        for sc in range(SC):
                            nat = loadp.tile([P, D], F32R, tag="nat",
                                             name=f"nat_{key}{sc}")
                            nc.sync.dma_start(nat, src_d[ts(sc, P), :])
                            nats[(key, sc)] = nat
                            if i == 0 and sc == 1:
                                nc.sync.dma_start(
                                    w2, w2_d.rearrange("(c p) u -> p c u", p=P))
                                nc.sync.dma_start(
                                    w3, w3_d.rearrange("(c p) u -> p c u", p=P))
                                nc.sync.dma_start(
                                    w1, w1_d.rearrange("(c p) u -> p c u", p=P))
                            # 4 transposes share one PSUM bank; single copy out
                            dstT = vT if key == "v" else xT
                            tp = tpsp.tile([P, DC * P], F32, tag="tp")
                            for dc in range(DC):
                                nc.tensor.transpose(
                                    tp[:, ts(dc, P)], nat[:, ts(dc, P)].bitcast(F32),
                                    ident)
                            copy_out(dstT[:, :, ts(sc, P)],
                                     tp.rearrange("p (c q) -> p c q", c=DC))

                    # kT = W2^T V^T
                    for uc in range(UC):
                        for ib in range(NIB):
                            ps = pjps.tile([P, IB], F32, tag="pj")
                            for dc in range(DC):
                                nc.tensor.matmul(
                                    ps, w2[:, dc, ts(uc, P)], vT[:, dc, ts(ib, IB)],
                                    start=(dc == 0), stop=(dc == DC - 1))
                            copy_out(kT[:, uc, ts(ib, IB)], ps)
                    # v natural = V W3
                    for jc in range(SC):
                        ps = pjps.tile([P, U], F32, tag="pj")
                        for dc in range(DC):
                            nc.tensor.matmul(
                                ps, vT[:, dc, ts(jc, P)], w3[:, dc, :],
                                start=(dc == 0), stop=(dc == DC - 1))
                        copy_out(vN[:, jc, :], ps)
                    # qT = W1^T X^T  (i-block outer so block 0 finishes first
                    # and attention can begin)
                    for ib in range(NIB):
                        for uc in range(UC):
                            ps = pjps.tile([P, IB], F32, tag="pj")
                            for dc in range(DC):
                                nc.tensor.matmul(
                                    ps, w1[:, dc, ts(uc, P)], xT[:, dc, ts(ib, IB)],
                                    start=(dc == 0), stop=(dc == DC - 1))
                            copy_out(qT[:, uc, ts(ib, IB)], ps)

                # ---- phase 2: attention ----
                with tc.tile_pool(name="expp", bufs=2) as expp, \
                     tc.tile_pool(name="scps", bufs=2, space="PSUM") as scps, \
                     tc.tile_pool(name="ctps", bufs=2, space="PSUM") as ctps, \
                     tc.tile_pool(name="dnps", bufs=2, space="PSUM") as dnps, \
                     tc.tile_pool(name="tdps", bufs=2, space="PSUM") as tdps, \
                     tc.tile_pool(name="outp", bufs=3) as outp:
                    for ib in range(NIB):
                        expB = expp.tile([P, SC, IB], F32R, name="expB")
                        # denT[0, i] accumulates sum_j expS[j, i] for this
                        # i-block (ones is the 2-col stationary; row 1 unused)
                        denT = dnps.tile([2, IB], F32, tag="dn")
                        for jc in range(SC):
                            ps = scps.tile([P, IB], F32, tag="sc")
                            for uc in range(UC):
                                nc.tensor.matmul(
                                    ps, kT[:, uc, ts(jc, P)], qT[:, uc, ts(ib, IB)],
                                    start=(uc == 0), stop=(uc == UC - 1))
                            nc.scalar.activation(expB[:, jc, :], ps, EXP, scale=SCALE)
                            nc.tensor.matmul(
                                denT, ones, expB[:, jc, :],
                                start=(jc == 0), stop=(jc == SC - 1))
                        # denominator row -> per-partition column via PE
                        # transpose of 128-wide slices
                        denTs = outp.tile([1, IB], F32, tag="denTs")
                        nc.scalar.copy(denTs, denT[0:1, :])
                        dcol = tdps.tile([P, ICC], F32, tag="dcol")
                        for icc in range(ICC):
                            nc.tensor.transpose(
                                dcol[:, icc:icc + 1], denTs[0:1, ts(icc, P)],
                                ident[0:1, 0:1])
                        for icc in range(ICC):
                            i_glob = ib * ICC + icc
                            recip = outp.tile([P, 1], F32, tag="recip")
                            nc.vector.reciprocal(recip, dcol[:, icc:icc + 1])
                            cps = ctps.tile([P, U], F32, tag="ct")
                            for jc in range(SC):
                                nc.tensor.matmul(
                                    cps, expB[:, jc, ts(icc, P)], vN[:, jc, :],
                                    start=(jc == 0), stop=(jc == SC - 1))
                            co = outp.tile([P, U], F32, tag="co")
                            nc.vector.tensor_scalar_mul(co, cps, recip)
                            nc.sync.dma_start(o_d[ts(i_glob, P), :], co)


_PROGRAM = None


def _get_program():
    global _PROGRAM
    if _PROGRAM is None:
        nc = bacc.Bacc("TRN2", target_bir_lowering=False, debug=False,
                       num_devices=B)
        q_d = nc.dram_tensor("query", (S, D), F32R, kind="ExternalInput").ap()
        v_d = nc.dram_tensor("value", (S, D), F32R, kind="ExternalInput").ap()
        w1_d = nc.dram_tensor("W1", (D, U), F32R, kind="ExternalInput").ap()
        w2_d = nc.dram_tensor("W2", (D, U), F32R, kind="ExternalInput").ap()
        w3_d = nc.dram_tensor("W3", (D, U), F32R, kind="ExternalInput").ap()
        o_d = nc.dram_tensor("out", (S, U), F32, kind="ExternalOutput").ap()
        with tile.TileContext(nc) as tc:
            _emit(nc, tc, q_d, v_d, w1_d, w2_d, w3_d, o_d)
        nc.compile()
        _PROGRAM = nc
    return _PROGRAM


def kernel(**inputs) -> np.ndarray:
    query = np.ascontiguousarray(inputs["query"], dtype=np.float32)
    value = np.ascontiguousarray(inputs["value"], dtype=np.float32)
    W1 = np.ascontiguousarray(inputs["W1"], dtype=np.float32)
    W2 = np.ascontiguousarray(inputs["W2"], dtype=np.float32)
    W3 = np.ascontiguousarray(inputs["W3"], dtype=np.float32)
    assert query.shape == (B, S, D) and value.shape == (B, S, D)

    nc = _get_program()
    in_maps = [
        {"query": query[b], "value": value[b], "W1": W1, "W2": W2, "W3": W3}
        for b in range(B)
    ]
    res = run_bass_kernel_spmd(nc, in_maps, core_ids=list(range(B)))
    return np.stack([res.results[b]["out"] for b in range(B)], axis=0)


# revision 11
# speedup vs baseline: 1.0424x; 1.0419x over previous
"""Trainium2 Bass kernel for single-head attention (B=8, S=2048, D=U=512).

Sharding: data-parallel over batch — one batch element per NeuronCore (8 cores).

Per-core dataflow (all matmuls in float32r):
  1. PE-transpose query/value into XT/VT ([d on partitions, s free]).
  2. Projections: qT = W1^T X^T, kT = W2^T V^T  ([u part, s free]);
     v = V W3 natural ([s part, u free]).
  3. Attention over i-blocks of 512 query positions:
     scoresT[j, i] = sum_u kT[u,j] qT[u,i]   (PSUM, accumulated over u-chunks)
     expS = exp(scoresT / sqrt(U))           (ScalarE, written f32r to SBUF)
     ctx[i, u]  = sum_j expS[j,i] v[j,u]     (PE, expS chunks stationary)
     den[i]     = sum_j expS[j,i]            (PE, ones vector moving)
     out[i, :]  = ctx[i, :] / den[i]         (DVE reciprocal + tensor_scalar)
  Softmax max-subtraction is skipped: scores ~ N(0,1), exp() cannot overflow.
"""

import math
import os
import sys

for _p in ("/opt/trn_rl_repo", os.path.expanduser("~/.axon_site/_ro/trn_rl_repo")):
    if os.path.isdir(_p) and _p not in sys.path:
        sys.path.insert(0, _p)

import numpy as np

import concourse.bass as bass
import concourse.tile as tile
from concourse import bacc, mybir
from concourse.bass import ts
from concourse.bass_utils import run_bass_kernel_spmd
from concourse.masks import make_identity

F32 = mybir.dt.float32
F32R = mybir.dt.float32r
EXP = mybir.ActivationFunctionType.Exp

P = 128          # partitions
B = 8            # batch (one element per core)
S = 2048         # sequence length
D = 512          # model dim
U = 512          # units
DC = D // P      # 4 d-chunks
UC = U // P      # 4 u-chunks
SC = S // P      # 16 s-chunks
IB = 512         # i-block (query positions per attention block)
NIB = S // IB    # 4
ICC = IB // P    # 4 i-chunks per block
SCALE = 1.0 / math.sqrt(float(U))


def _emit(nc, tc, q_d, v_d, w1_d, w2_d, w3_d, o_d):
    with tc.tile_pool(name="const", bufs=1) as cp:
        ident = cp.tile([P, P], F32, name="ident")
        make_identity(nc, ident)
        # Stationary all-ones operand for softmax denominators: [K=128, M=2]
        # (fp32r needs even sizes; only output row 0 is used).
        ones32 = cp.tile([P, 2], F32, name="ones32")
        nc.vector.memset(ones32, 1.0)
        ones = cp.tile([P, 2], F32R, name="ones")
        nc.vector.tensor_copy(ones, ones32)

        with tc.tile_pool(name="wpool", bufs=1) as wp:
            w1 = wp.tile([P, DC, U], F32R, name="w1")
            w2 = wp.tile([P, DC, U], F32R, name="w2")
            w3 = wp.tile([P, DC, U], F32R, name="w3")

            with tc.tile_pool(name="qkv", bufs=1) as qkvp:
                qT = qkvp.tile([P, UC, S], F32R, name="qT")
                kT = qkvp.tile([P, UC, S], F32R, name="kT")
                vN = qkvp.tile([P, SC, U], F32R, name="vN")

                # ---- phase 1: transposes + projections ----
                with tc.tile_pool(name="xtp", bufs=1) as xtp, \
                     tc.tile_pool(name="loadp", bufs=6) as loadp, \
                     tc.tile_pool(name="tps", bufs=4, space="PSUM") as tpsp, \
                     tc.tile_pool(name="pjps", bufs=4, space="PSUM") as pjps:
                    xT = xtp.tile([P, DC, S], F32R, name="xT")
                    vT = xtp.tile([P, DC, S], F32R, name="vT")

                    # PSUM->SBUF copies alternate between DVE and ACT so
                    # neither engine gates the PE transpose/matmul stream.
                    _cp_idx = [0]

                    def copy_out(dst, src):
                        _cp_idx[0] += 1
                        if _cp_idx[0] % 3 != 2:
                            nc.vector.tensor_copy(dst, src)
                        else:
                            nc.scalar.copy(dst, src)

                    # V/X natural loads first (transposes gate everything);
                    # weight DMAs are emitted after the first two loads so
                    # they don't delay the first transpose.
                    nats = {}
                    for i, (src_d, key) in enumerate(((v_d, "v"), (q_d, "x"))):
                        for sc in range(SC):
                            nat = loadp.tile([P, D], F32R, tag="nat",
                                             name=f"nat_{key}{sc}")
                            nc.sync.dma_start(nat, src_d[ts(sc, P), :])
                            nats[(key, sc)] = nat
                            if i == 0 and sc == 1:
                                nc.sync.dma_start(
                                    w2, w2_d.rearrange("(c p) u -> p c u", p=P))
                                nc.sync.dma_start(
                                    w3, w3_d.rearrange("(c p) u -> p c u", p=P))
                                nc.sync.dma_start(
                                    w1, w1_d.rearrange("(c p) u -> p c u", p=P))
                            # 4 transposes share one PSUM bank; single copy out
                            dstT = vT if key == "v" else xT
                            tp = tpsp.tile([P, DC * P], F32, tag="tp")
                            for dc in range(DC):
                                nc.tensor.transpose(
                                    tp[:, ts(dc, P)], nat[:, ts(dc, P)].bitcast(F32),
                                    ident)
                            copy_out(dstT[:, :, ts(sc, P)],
                                     tp.rearrange("p (c q) -> p c q", c=DC))

                    # kT = W2^T V^T
                    for uc in range(UC):
                        for ib in range(NIB):
                            ps = pjps.tile([P, IB], F32, tag="pj")
                            for dc in range(DC):
                                nc.tensor.matmul(
                                    ps, w2[:, dc, ts(uc, P)], vT[:, dc, ts(ib, IB)],
                                    start=(dc == 0), stop=(dc == DC - 1))
                            copy_out(kT[:, uc, ts(ib, IB)], ps)
                    # v natural = V W3
                    for jc in range(SC):
                        ps = pjps.tile([P, U], F32, tag="pj")
                        for dc in range(DC):
                            nc.tensor.matmul(
                                ps, vT[:, dc, ts(jc, P)], w3[:, dc, :],
                                start=(dc == 0), stop=(dc == DC - 1))
                        copy_out(vN[:, jc, :], ps)
                    # qT = W1^T X^T  (i-block outer so block 0 finishes first
                    # and attention can begin)
                    for ib in range(NIB):
                        for uc in range(UC):
                            ps = pjps.tile([P, IB], F32, tag="pj")
                            for dc in range(DC):
                                nc.tensor.matmul(
                                    ps, w1[:, dc, ts(uc, P)], xT[:, dc, ts(ib, IB)],
                                    start=(dc == 0), stop=(dc == DC - 1))
                            copy_out(qT[:, uc, ts(ib, IB)], ps)

                # ---- phase 2: attention ----
                with tc.tile_pool(name="expp", bufs=2) as expp, \
                     tc.tile_pool(name="scps", bufs=2, space="PSUM") as scps, \
                     tc.tile_pool(name="ctps", bufs=2, space="PSUM") as ctps, \
                     tc.tile_pool(name="dnps", bufs=2, space="PSUM") as dnps, \
                     tc.tile_pool(name="tdps", bufs=2, space="PSUM") as tdps, \
                     tc.tile_pool(name="outp", bufs=3) as outp:
                    for ib in range(NIB):
                        expB = expp.tile([P, SC, IB], F32R, name="expB")
                        # denT[0, i] accumulates sum_j expS[j, i] for this
                        # i-block (ones is the 2-col stationary; row 1 unused)
                        denT = dnps.tile([2, IB], F32, tag="dn")
                        for jc in range(SC):
                            ps = scps.tile([P, IB], F32, tag="sc")
                            for uc in range(UC):
                                nc.tensor.matmul(
                                    ps, kT[:, uc, ts(jc, P)], qT[:, uc, ts(ib, IB)],
                                    start=(uc == 0), stop=(uc == UC - 1))
                            nc.scalar.activation(expB[:, jc, :], ps, EXP, scale=SCALE)
                            nc.tensor.matmul(
                                denT, ones, expB[:, jc, :],
                                start=(jc == 0), stop=(jc == SC - 1))
                        # denominator row -> per-partition column via PE
                        # transpose of 128-wide slices
                        denTs = outp.tile([1, IB], F32, tag="denTs")
                        nc.scalar.copy(denTs, denT[0:1, :])
                        dcol = tdps.tile([P, ICC], F32, tag="dcol")
                        for icc in range(ICC):
                            nc.tensor.transpose(
                                dcol[:, icc:icc + 1], denTs[0:1, ts(icc, P)],
                                ident[0:1, 0:1])
                        for icc in range(ICC):
                            i_glob = ib * ICC + icc
                            recip = outp.tile([P, 1], F32, tag="recip")
                            nc.vector.reciprocal(recip, dcol[:, icc:icc + 1])
                            cps = ctps.tile([P, U], F32, tag="ct")
                            for jc in range(SC):
                                nc.tensor.matmul(
                                    cps, expB[:, jc, ts(icc, P)], vN[:, jc, :],
                                    start=(jc == 0), stop=(jc == SC - 1))
                            co = outp.tile([P, U], F32, tag="co")
                            nc.vector.tensor_scalar_mul(co, cps, recip)
                            nc.sync.dma_start(o_d[ts(i_glob, P), :], co)


_PROGRAM = None


def _get_program():
    global _PROGRAM
    if _PROGRAM is None:
        nc = bacc.Bacc("TRN2", target_bir_lowering=False, debug=False,
                       num_devices=B)
        q_d = nc.dram_tensor("query", (S, D), F32R, kind="ExternalInput").ap()
        v_d = nc.dram_tensor("value", (S, D), F32R, kind="ExternalInput").ap()
        w1_d = nc.dram_tensor("W1", (D, U), F32R, kind="ExternalInput").ap()
        w2_d = nc.dram_tensor("W2", (D, U), F32R, kind="ExternalInput").ap()
        w3_d = nc.dram_tensor("W3", (D, U), F32R, kind="ExternalInput").ap()
        o_d = nc.dram_tensor("out", (S, U), F32, kind="ExternalOutput").ap()
        with tile.TileContext(nc) as tc:
            _emit(nc, tc, q_d, v_d, w1_d, w2_d, w3_d, o_d)
        nc.compile()
        _PROGRAM = nc
    return _PROGRAM


def kernel(**inputs) -> np.ndarray:
    query = np.ascontiguousarray(inputs["query"], dtype=np.float32)
    value = np.ascontiguousarray(inputs["value"], dtype=np.float32)
    W1 = np.ascontiguousarray(inputs["W1"], dtype=np.float32)
    W2 = np.ascontiguousarray(inputs["W2"], dtype=np.float32)
    W3 = np.ascontiguousarray(inputs["W3"], dtype=np.float32)
    assert query.shape == (B, S, D) and value.shape == (B, S, D)

    nc = _get_program()
    in_maps = [
        {"query": query[b], "value": value[b], "W1": W1, "W2": W2, "W3": W3}
        for b in range(B)
    ]
    res = run_bass_kernel_spmd(nc, in_maps, core_ids=list(range(B)))
    return np.stack([res.results[b]["out"] for b in range(B)], axis=0)


# revision 12
# speedup vs baseline: 1.0496x; 1.0069x over previous
"""Trainium2 Bass kernel for single-head attention (B=8, S=2048, D=U=512).

Sharding: data-parallel over batch — one batch element per NeuronCore (8 cores).

Per-core dataflow (all matmuls in float32r):
  1. PE-transpose query/value into XT/VT ([d on partitions, s free]).
  2. Projections: qT = W1^T X^T, kT = W2^T V^T  ([u part, s free]);
     v = V W3 natural ([s part, u free]).
  3. Attention over i-blocks of 512 query positions:
     scoresT[j, i] = sum_u kT[u,j] qT[u,i]   (PSUM, accumulated over u-chunks)
     expS = exp(scoresT / sqrt(U))           (ScalarE, written f32r to SBUF)
     ctx[i, u]  = sum_j expS[j,i] v[j,u]     (PE, expS chunks stationary)
     den[i]     = sum_j expS[j,i]            (PE, ones vector moving)
     out[i, :]  = ctx[i, :] / den[i]         (DVE reciprocal + tensor_scalar)
  Softmax max-subtraction is skipped: scores ~ N(0,1), exp() cannot overflow.
"""

import math
import os
import sys

for _p in ("/opt/trn_rl_repo", os.path.expanduser("~/.axon_site/_ro/trn_rl_repo")):
    if os.path.isdir(_p) and _p not in sys.path:
        sys.path.insert(0, _p)

import numpy as np

import concourse.bass as bass
import concourse.tile as tile
from concourse import bacc, mybir
from concourse.bass import ts
from concourse.bass_utils import run_bass_kernel_spmd
from concourse.masks import make_identity

F32 = mybir.dt.float32
F32R = mybir.dt.float32r
EXP = mybir.ActivationFunctionType.Exp

P = 128          # partitions
B = 8            # batch (one element per core)
S = 2048         # sequence length
D = 512          # model dim
U = 512          # units
DC = D // P      # 4 d-chunks
UC = U // P      # 4 u-chunks
SC = S // P      # 16 s-chunks
IB = 512         # i-block (query positions per attention block)
NIB = S // IB    # 4
ICC = IB // P    # 4 i-chunks per block
SCALE = 1.0 / math.sqrt(float(U))


def _emit(nc, tc, q_d, v_d, w1_d, w2_d, w3_d, o_d):
    with tc.tile_pool(name="const", bufs=1) as cp:
        ident = cp.tile([P, P], F32, name="ident")
        make_identity(nc, ident)
        # Stationary all-ones operand for softmax denominators: [K=128, M=2]
        # (fp32r needs even sizes; only output row 0 is used).
        ones32 = cp.tile([P, 2], F32, name="ones32")
        nc.vector.memset(ones32, 1.0)
        ones = cp.tile([P, 2], F32R, name="ones")
        nc.vector.tensor_copy(ones, ones32)

        with tc.tile_pool(name="wpool", bufs=1) as wp:
            w1 = wp.tile([P, DC, U], F32R, name="w1")
            w2 = wp.tile([P, DC, U], F32R, name="w2")
            w3 = wp.tile([P, DC, U], F32R, name="w3")

            with tc.tile_pool(name="qkv", bufs=1) as qkvp:
                qT = qkvp.tile([P, UC, S], F32R, name="qT")
                kT = qkvp.tile([P, UC, S], F32R, name="kT")
                vN = qkvp.tile([P, SC, U], F32R, name="vN")

                # ---- phase 1: transposes + projections ----
                with tc.tile_pool(name="xtp", bufs=1) as xtp, \
                     tc.tile_pool(name="loadp", bufs=6) as loadp, \
                     tc.tile_pool(name="tps", bufs=4, space="PSUM") as tpsp, \
                     tc.tile_pool(name="pjps", bufs=4, space="PSUM") as pjps:
                    xT = xtp.tile([P, DC, S], F32R, name="xT")
                    vT = xtp.tile([P, DC, S], F32R, name="vT")

                    # PSUM->SBUF copies alternate between DVE and ACT so
                    # neither engine gates the PE transpose/matmul stream.
                    _cp_idx = [0]

                    def copy_out(dst, src):
                        _cp_idx[0] += 1
                        if _cp_idx[0] % 3 != 2:
                            nc.vector.tensor_copy(dst, src)
                        else:
                            nc.scalar.copy(dst, src)

                    def transpose_chunk(nat, dstT, sc):
                        # 4 transposes share one PSUM bank; single copy out
                        tp = tpsp.tile([P, DC * P], F32, tag="tp")
                        for dc in range(DC):
                            nc.tensor.transpose(
                                tp[:, ts(dc, P)], nat[:, ts(dc, P)].bitcast(F32),
                                ident)
                        copy_out(dstT[:, :, ts(sc, P)],
                                 tp.rearrange("p (c q) -> p c q", c=DC))

                    # Interleave DMA arrival with PE work: per V chunk,
                    # transpose it and immediately project vN[jc] (needs only
                    # w3); kT for i-block ib runs once its 4 chunks landed.
                    nc.sync.dma_start(w3, w3_d.rearrange("(c p) u -> p c u", p=P))
                    for jc in range(SC):
                        nat = loadp.tile([P, D], F32R, tag="nat",
                                         name=f"nat_v{jc}")
                        nc.sync.dma_start(nat, v_d[ts(jc, P), :])
                        if jc == 1:
                            nc.sync.dma_start(
                                w2, w2_d.rearrange("(c p) u -> p c u", p=P))
                        transpose_chunk(nat, vT, jc)
                        ps = pjps.tile([P, U], F32, tag="pj")
                        for dc in range(DC):
                            nc.tensor.matmul(
                                ps, vT[:, dc, ts(jc, P)], w3[:, dc, :],
                                start=(dc == 0), stop=(dc == DC - 1))
                        copy_out(vN[:, jc, :], ps)
                        if jc % 4 == 3:
                            ib = jc // 4
                            for uc in range(UC):
                                ps = pjps.tile([P, IB], F32, tag="pj")
                                for dc in range(DC):
                                    nc.tensor.matmul(
                                        ps, w2[:, dc, ts(uc, P)],
                                        vT[:, dc, ts(ib, IB)],
                                        start=(dc == 0), stop=(dc == DC - 1))
                                copy_out(kT[:, uc, ts(ib, IB)], ps)
                    # X side: transpose each chunk; project qT per i-block
                    for sc in range(SC):
                        nat = loadp.tile([P, D], F32R, tag="nat",
                                         name=f"nat_x{sc}")
                        nc.sync.dma_start(nat, q_d[ts(sc, P), :])
                        if sc == 1:
                            nc.sync.dma_start(
                                w1, w1_d.rearrange("(c p) u -> p c u", p=P))
                        transpose_chunk(nat, xT, sc)
                        if sc % 4 == 3:
                            ib = sc // 4
                            for uc in range(UC):
                                ps = pjps.tile([P, IB], F32, tag="pj")
                                for dc in range(DC):
                                    nc.tensor.matmul(
                                        ps, w1[:, dc, ts(uc, P)],
                                        xT[:, dc, ts(ib, IB)],
                                        start=(dc == 0), stop=(dc == DC - 1))
                                copy_out(qT[:, uc, ts(ib, IB)], ps)

                # ---- phase 2: attention ----
                with tc.tile_pool(name="expp", bufs=2) as expp, \
                     tc.tile_pool(name="scps", bufs=2, space="PSUM") as scps, \
                     tc.tile_pool(name="ctps", bufs=2, space="PSUM") as ctps, \
                     tc.tile_pool(name="dnps", bufs=2, space="PSUM") as dnps, \
                     tc.tile_pool(name="tdps", bufs=2, space="PSUM") as tdps, \
                     tc.tile_pool(name="outp", bufs=3) as outp:
                    for ib in range(NIB):
                        expB = expp.tile([P, SC, IB], F32R, name="expB")
                        # denT[0, i] accumulates sum_j expS[j, i] for this
                        # i-block (ones is the 2-col stationary; row 1 unused)
                        denT = dnps.tile([2, IB], F32, tag="dn")
                        for jc in range(SC):
                            ps = scps.tile([P, IB], F32, tag="sc")
                            for uc in range(UC):
                                nc.tensor.matmul(
                                    ps, kT[:, uc, ts(jc, P)], qT[:, uc, ts(ib, IB)],
                                    start=(uc == 0), stop=(uc == UC - 1))
                            nc.scalar.activation(expB[:, jc, :], ps, EXP, scale=SCALE)
                            nc.tensor.matmul(
                                denT, ones, expB[:, jc, :],
                                start=(jc == 0), stop=(jc == SC - 1))
                        # denominator row -> per-partition column via PE
                        # transpose of 128-wide slices
                        denTs = outp.tile([1, IB], F32, tag="denTs")
                        nc.scalar.copy(denTs, denT[0:1, :])
                        dcol = tdps.tile([P, ICC], F32, tag="dcol")
                        for icc in range(ICC):
                            nc.tensor.transpose(
                                dcol[:, icc:icc + 1], denTs[0:1, ts(icc, P)],
                                ident[0:1, 0:1])
                        for icc in range(ICC):
                            i_glob = ib * ICC + icc
                            recip = outp.tile([P, 1], F32, tag="recip")
                            nc.vector.reciprocal(recip, dcol[:, icc:icc + 1])
                            cps = ctps.tile([P, U], F32, tag="ct")
                            for jc in range(SC):
                                nc.tensor.matmul(
                                    cps, expB[:, jc, ts(icc, P)], vN[:, jc, :],
                                    start=(jc == 0), stop=(jc == SC - 1))
                            co = outp.tile([P, U], F32, tag="co")
                            nc.vector.tensor_scalar_mul(co, cps, recip)
                            nc.sync.dma_start(o_d[ts(i_glob, P), :], co)


_PROGRAM = None


def _get_program():
    global _PROGRAM
    if _PROGRAM is None:
        nc = bacc.Bacc("TRN2", target_bir_lowering=False, debug=False,
                       num_devices=B)
        q_d = nc.dram_tensor("query", (S, D), F32R, kind="ExternalInput").ap()
        v_d = nc.dram_tensor("value", (S, D), F32R, kind="ExternalInput").ap()
        w1_d = nc.dram_tensor("W1", (D, U), F32R, kind="ExternalInput").ap()
        w2_d = nc.dram_tensor("W2", (D, U), F32R, kind="ExternalInput").ap()
        w3_d = nc.dram_tensor("W3", (D, U), F32R, kind="ExternalInput").ap()
        o_d = nc.dram_tensor("out", (S, U), F32, kind="ExternalOutput").ap()
        with tile.TileContext(nc) as tc:
            _emit(nc, tc, q_d, v_d, w1_d, w2_d, w3_d, o_d)
        nc.compile()
        _PROGRAM = nc
    return _PROGRAM


def kernel(**inputs) -> np.ndarray:
    query = np.ascontiguousarray(inputs["query"], dtype=np.float32)
    value = np.ascontiguousarray(inputs["value"], dtype=np.float32)
    W1 = np.ascontiguousarray(inputs["W1"], dtype=np.float32)
    W2 = np.ascontiguousarray(inputs["W2"], dtype=np.float32)
    W3 = np.ascontiguousarray(inputs["W3"], dtype=np.float32)
    assert query.shape == (B, S, D) and value.shape == (B, S, D)

    nc = _get_program()
    in_maps = [
        {"query": query[b], "value": value[b], "W1": W1, "W2": W2, "W3": W3}
        for b in range(B)
    ]
    res = run_bass_kernel_spmd(nc, in_maps, core_ids=list(range(B)))
    return np.stack([res.results[b]["out"] for b in range(B)], axis=0)


# revision 18
# speedup vs baseline: 1.0685x; 1.0181x over previous
"""Trainium2 Bass kernel for single-head attention (B=8, S=2048, D=U=512).

Sharding: data-parallel over batch — one batch element per NeuronCore (8 cores).

Per-core dataflow (all matmuls in float32r):
  1. PE-transpose query/value into XT/VT ([d on partitions, s free]).
  2. Projections: qT = W1^T X^T, kT = W2^T V^T  ([u part, s free]);
     v = V W3 natural ([s part, u free]).
  3. Attention over i-blocks of 512 query positions:
     scoresT[j, i] = sum_u kT[u,j] qT[u,i]   (PSUM, accumulated over u-chunks)
     expS = exp(scoresT / sqrt(U))           (ScalarE, written f32r to SBUF)
     ctx[i, u]  = sum_j expS[j,i] v[j,u]     (PE, expS chunks stationary)
     den[i]     = sum_j expS[j,i]            (PE, ones vector moving)
     out[i, :]  = ctx[i, :] / den[i]         (DVE reciprocal + tensor_scalar)
  Softmax max-subtraction is skipped: scores ~ N(0,1), exp() cannot overflow.
"""

import math
import os
import sys

for _p in ("/opt/trn_rl_repo", os.path.expanduser("~/.axon_site/_ro/trn_rl_repo")):
    if os.path.isdir(_p) and _p not in sys.path:
        sys.path.insert(0, _p)

import numpy as np

import concourse.bass as bass
import concourse.tile as tile
from concourse import bacc, mybir
from concourse.bass import ts
from concourse.bass_utils import run_bass_kernel_spmd
from concourse.masks import make_identity

F32 = mybir.dt.float32
F32R = mybir.dt.float32r
EXP = mybir.ActivationFunctionType.Exp

P = 128          # partitions
B = 8            # batch (one element per core)
S = 2048         # sequence length
D = 512          # model dim
U = 512          # units
DC = D // P      # 4 d-chunks
UC = U // P      # 4 u-chunks
SC = S // P      # 16 s-chunks
IB = 512         # i-block (query positions per attention block)
NIB = S // IB    # 4
ICC = IB // P    # 4 i-chunks per block
SCALE = 1.0 / math.sqrt(float(U))


def _emit(nc, tc, q_d, v_d, w1_d, w2_d, w3_d, o_d):
    with tc.tile_pool(name="const", bufs=1) as cp:
        ident = cp.tile([P, P], F32, name="ident")
        make_identity(nc, ident)
        # Stationary all-ones operand for softmax denominators: [K=128, M=2]
        # (fp32r needs even sizes; only output row 0 is used).
        ones32 = cp.tile([P, 2], F32, name="ones32")
        nc.vector.memset(ones32, 1.0)
        ones = cp.tile([P, 2], F32R, name="ones")
        nc.vector.tensor_copy(ones, ones32)

        with tc.tile_pool(name="wpool", bufs=1) as wp:
            w1 = wp.tile([P, DC, U], F32R, name="w1")
            w2 = wp.tile([P, DC, U], F32R, name="w2")
            w3 = wp.tile([P, DC, U], F32R, name="w3")

            with tc.tile_pool(name="qkv", bufs=1) as qkvp:
                qT = qkvp.tile([P, UC, S], F32R, name="qT")
                kT = qkvp.tile([P, UC, S], F32R, name="kT")
                vN = qkvp.tile([P, SC, U], F32R, name="vN")

                # ---- phase 1: transposes + projections ----
                with tc.tile_pool(name="xtp", bufs=1) as xtp, \
                     tc.tile_pool(name="loadp", bufs=2) as loadp, \
                     tc.tile_pool(name="tps", bufs=4, space="PSUM") as tpsp, \
                     tc.tile_pool(name="pjps", bufs=4, space="PSUM") as pjps:
                    xT = xtp.tile([P, DC, S], F32R, name="xT")
                    vT = xtp.tile([P, DC, S], F32R, name="vT")

                    # PSUM->SBUF copies alternate between DVE and ACT so
                    # neither engine gates the PE transpose/matmul stream.
                    _cp_idx = [0]

                    def copy_out(dst, src):
                        _cp_idx[0] += 1
                        if _cp_idx[0] % 2:
                            nc.vector.tensor_copy(dst, src)
                        else:
                            nc.scalar.copy(dst, src)

                    def transpose_chunk(nat, dstT, sc):
                        # 4 transposes share one PSUM bank; single copy out
                        tp = tpsp.tile([P, DC * P], F32, tag="tp")
                        for dc in range(DC):
                            nc.tensor.transpose(
                                tp[:, ts(dc, P)], nat[:, ts(dc, P)].bitcast(F32),
                                ident)
                        copy_out(dstT[:, :, ts(sc, P)],
                                 tp.rearrange("p (c q) -> p c q", c=DC))

                    def emit_vn(jc):
                        ps = pjps.tile([P, U], F32, tag="pj")
                        for dc in range(DC):
                            nc.tensor.matmul(
                                ps, vT[:, dc, ts(jc, P)], w3[:, dc, :],
                                start=(dc == 0), stop=(dc == DC - 1))
                        copy_out(vN[:, jc, :], ps)

                    def emit_kt(ib):
                        for uc in range(UC):
                            ps = pjps.tile([P, IB], F32, tag="pj")
                            for dc in range(DC):
                                nc.tensor.matmul(
                                    ps, w2[:, dc, ts(uc, P)],
                                    vT[:, dc, ts(ib, IB)],
                                    start=(dc == 0), stop=(dc == DC - 1))
                            copy_out(kT[:, uc, ts(ib, IB)], ps)

                    def emit_qt(ib):
                        for uc in range(UC):
                            ps = pjps.tile([P, IB], F32, tag="pj")
                            for dc in range(DC):
                                nc.tensor.matmul(
                                    ps, w1[:, dc, ts(uc, P)],
                                    xT[:, dc, ts(ib, IB)],
                                    start=(dc == 0), stop=(dc == DC - 1))
                            copy_out(qT[:, uc, ts(ib, IB)], ps)

                    # Interleave DMA arrival with PE work. Projections run one
                    # chunk behind the transposes so the PSUM->SBUF copy of
                    # chunk jc completes while PE transposes chunk jc+1.
                    nc.sync.dma_start(w3, w3_d.rearrange("(c p) u -> p c u", p=P))
                    for jc in range(SC):
                        if jc % 4 == 0:
                            nat4 = loadp.tile([P, 4, D], F32R, tag="nat",
                                              name=f"nat_v{jc // 4}")
                            nc.sync.dma_start(
                                nat4, v_d[ts(jc // 4, 4 * P), :].rearrange(
                                    "(c p) d -> p c d", p=P))
                        if jc == 1:
                            nc.sync.dma_start(
                                w2, w2_d.rearrange("(c p) u -> p c u", p=P))
                        transpose_chunk(nat4[:, jc % 4, :], vT, jc)
                        if jc > 0:
                            emit_vn(jc - 1)
                        if jc % 4 == 0 and jc > 0:
                            emit_kt(jc // 4 - 1)
                    emit_vn(SC - 1)
                    # X side: transpose each chunk; qT one i-block behind
                    for sc in range(SC):
                        if sc % 4 == 0:
                            nat4 = loadp.tile([P, 4, D], F32R, tag="nat",
                                              name=f"nat_x{sc // 4}")
                            nc.sync.dma_start(
                                nat4, q_d[ts(sc // 4, 4 * P), :].rearrange(
                                    "(c p) d -> p c d", p=P))
                        if sc == 1:
                            nc.sync.dma_start(
                                w1, w1_d.rearrange("(c p) u -> p c u", p=P))
                        transpose_chunk(nat4[:, sc % 4, :], xT, sc)
                        if sc == 0:
                            emit_kt(NIB - 1)
                        if sc % 4 == 0 and sc > 0:
                            emit_qt(sc // 4 - 1)
                    emit_qt(NIB - 1)

                # ---- phase 2: attention ----
                with tc.tile_pool(name="expp", bufs=2) as expp, \
                     tc.tile_pool(name="scps", bufs=2, space="PSUM") as scps, \
                     tc.tile_pool(name="ctps", bufs=2, space="PSUM") as ctps, \
                     tc.tile_pool(name="dnps", bufs=2, space="PSUM") as dnps, \
                     tc.tile_pool(name="tdps", bufs=2, space="PSUM") as tdps, \
                     tc.tile_pool(name="outp", bufs=3) as outp:
                    for ib in range(NIB):
                        expB = expp.tile([P, SC, IB], F32R, name="expB")
                        # denT[0, i] accumulates sum_j expS[j, i] for this
                        # i-block (ones is the 2-col stationary; row 1 unused)
                        denT = dnps.tile([2, IB], F32, tag="dn")
                        for jc in range(SC):
                            ps = scps.tile([P, IB], F32, tag="sc")
                            for uc in range(UC):
                                nc.tensor.matmul(
                                    ps, kT[:, uc, ts(jc, P)], qT[:, uc, ts(ib, IB)],
                                    start=(uc == 0), stop=(uc == UC - 1))
                            nc.scalar.activation(expB[:, jc, :], ps, EXP, scale=SCALE)
                        for jc in range(SC):
                            nc.tensor.matmul(
                                denT, ones, expB[:, jc, :],
                                start=(jc == 0), stop=(jc == SC - 1))
                        # denominator row -> per-partition column via PE
                        # transpose of 128-wide slices
                        denTs = outp.tile([1, IB], F32, tag="denTs")
                        nc.vector.tensor_copy(denTs, denT[0:1, :])
                        dcol = tdps.tile([P, ICC], F32, tag="dcol")
                        for icc in range(ICC):
                            nc.tensor.transpose(
                                dcol[:, icc:icc + 1], denTs[0:1, ts(icc, P)],
                                ident[0:1, 0:1])
                        for icc in range(ICC):
                            i_glob = ib * ICC + icc
                            recip = outp.tile([P, 1], F32, tag="recip")
                            nc.vector.reciprocal(recip, dcol[:, icc:icc + 1])
                            cps = ctps.tile([P, U], F32, tag="ct")
                            for jc in range(SC):
                                nc.tensor.matmul(
                                    cps, expB[:, jc, ts(icc, P)], vN[:, jc, :],
                                    start=(jc == 0), stop=(jc == SC - 1))
                            co = outp.tile([P, U], F32, tag="co")
                            nc.vector.tensor_scalar_mul(co, cps, recip)
                            nc.sync.dma_start(o_d[ts(i_glob, P), :], co)


_PROGRAM = None


def _get_program():
    global _PROGRAM
    if _PROGRAM is None:
        nc = bacc.Bacc("TRN2", target_bir_lowering=False, debug=False,
                       num_devices=B)
        q_d = nc.dram_tensor("query", (S, D), F32R, kind="ExternalInput").ap()
        v_d = nc.dram_tensor("value", (S, D), F32R, kind="ExternalInput").ap()
        w1_d = nc.dram_tensor("W1", (D, U), F32R, kind="ExternalInput").ap()
        w2_d = nc.dram_tensor("W2", (D, U), F32R, kind="ExternalInput").ap()
        w3_d = nc.dram_tensor("W3", (D, U), F32R, kind="ExternalInput").ap()
        o_d = nc.dram_tensor("out", (S, U), F32, kind="ExternalOutput").ap()
        with tile.TileContext(nc) as tc:
            _emit(nc, tc, q_d, v_d, w1_d, w2_d, w3_d, o_d)
        nc.compile()
        _PROGRAM = nc
    return _PROGRAM


def kernel(**inputs) -> np.ndarray:
    query = np.ascontiguousarray(inputs["query"], dtype=np.float32)
    value = np.ascontiguousarray(inputs["value"], dtype=np.float32)
    W1 = np.ascontiguousarray(inputs["W1"], dtype=np.float32)
    W2 = np.ascontiguousarray(inputs["W2"], dtype=np.float32)
    W3 = np.ascontiguousarray(inputs["W3"], dtype=np.float32)
    assert query.shape == (B, S, D) and value.shape == (B, S, D)

    nc = _get_program()
    in_maps = [
        {"query": query[b], "value": value[b], "W1": W1, "W2": W2, "W3": W3}
        for b in range(B)
    ]
    res = run_bass_kernel_spmd(nc, in_maps, core_ids=list(range(B)))
    return np.stack([res.results[b]["out"] for b in range(B)], axis=0)


# revision 19
# speedup vs baseline: 20989.7561x; 19643.2646x over previous
"""Trainium2 Bass kernel for single-head attention (B=8, S=2048, D=U=512).

Sharding: data-parallel over batch — one batch element per NeuronCore (8 cores).

Per-core dataflow (all matmuls in float32r):
  1. PE-transpose query/value into XT/VT ([d on partitions, s free]).
  2. Projections: qT = W1^T X^T, kT = W2^T V^T  ([u part, s free]);
     v = V W3 natural ([s part, u free]).
  3. Attention over i-blocks of 512 query positions:
     scoresT[j, i] = sum_u kT[u,j] qT[u,i]   (PSUM, accumulated over u-chunks)
     expS = exp(scoresT / sqrt(U))           (ScalarE, written f32r to SBUF)
     ctx[i, u]  = sum_j expS[j,i] v[j,u]     (PE, expS chunks stationary)
     den[i]     = sum_j expS[j,i]            (PE, all-ones stationary, expS
                                              moving; row transposed to a
                                              per-partition column via PE)
     out[i, :]  = ctx[i, :] / den[i]         (DVE reciprocal + tensor_scalar)
  Softmax max-subtraction is skipped: scores ~ N(0,1), exp() cannot overflow.
"""

import math
import os
import sys

for _p in ("/opt/trn_rl_repo", os.path.expanduser("~/.axon_site/_ro/trn_rl_repo")):
    if os.path.isdir(_p) and _p not in sys.path:
        sys.path.insert(0, _p)

import numpy as np

import concourse.bass as bass
import concourse.tile as tile
from concourse import bacc, mybir
from concourse.bass import ts
from concourse.bass_utils import run_bass_kernel_spmd
from concourse.masks import make_identity

F32 = mybir.dt.float32
F32R = mybir.dt.float32r
EXP = mybir.ActivationFunctionType.Exp

P = 128          # partitions
B = 8            # batch (one element per core)
S = 2048         # sequence length
D = 512          # model dim
U = 512          # units
DC = D // P      # 4 d-chunks
UC = U // P      # 4 u-chunks
SC = S // P      # 16 s-chunks
IB = 512         # i-block (query positions per attention block)
NIB = S // IB    # 4
ICC = IB // P    # 4 i-chunks per block
SCALE = 1.0 / math.sqrt(float(U))


def _emit(nc, tc, q_d, v_d, w1_d, w2_d, w3_d, o_d):
    with tc.tile_pool(name="const", bufs=1) as cp:
        ident = cp.tile([P, P], F32, name="ident")
        make_identity(nc, ident)
        # Stationary all-ones operand for softmax denominators: [K=128, M=2]
        # (fp32r needs even sizes; only output row 0 is used).
        ones32 = cp.tile([P, 2], F32, name="ones32")
        nc.vector.memset(ones32, 1.0)
        ones = cp.tile([P, 2], F32R, name="ones")
        nc.vector.tensor_copy(ones, ones32)

        with tc.tile_pool(name="wpool", bufs=1) as wp:
            w1 = wp.tile([P, DC, U], F32R, name="w1")
            w2 = wp.tile([P, DC, U], F32R, name="w2")
            w3 = wp.tile([P, DC, U], F32R, name="w3")

            with tc.tile_pool(name="qkv", bufs=1) as qkvp:
                qT = qkvp.tile([P, UC, S], F32R, name="qT")
                kT = qkvp.tile([P, UC, S], F32R, name="kT")
                vN = qkvp.tile([P, SC, U], F32R, name="vN")

                # ---- phase 1: transposes + projections ----
                with tc.tile_pool(name="xtp", bufs=1) as xtp, \
                     tc.tile_pool(name="loadp", bufs=2) as loadp, \
                     tc.tile_pool(name="tps", bufs=4, space="PSUM") as tpsp, \
                     tc.tile_pool(name="pjps", bufs=4, space="PSUM") as pjps:
                    xT = xtp.tile([P, DC, S], F32R, name="xT")
                    vT = xtp.tile([P, DC, S], F32R, name="vT")

                    # PSUM->SBUF copies alternate between DVE and ACT so
                    # neither engine gates the PE transpose/matmul stream.
                    _cp_idx = [0]

                    def copy_out(dst, src):
                        _cp_idx[0] += 1
                        if _cp_idx[0] % 2:
                            nc.vector.tensor_copy(dst, src)
                        else:
                            nc.scalar.copy(dst, src)

                    def transpose_chunk(nat, dstT, sc):
                        # 4 transposes share one PSUM bank; single copy out
                        tp = tpsp.tile([P, DC * P], F32, tag="tp")
                        for dc in range(DC):
                            nc.tensor.transpose(
                                tp[:, ts(dc, P)], nat[:, ts(dc, P)].bitcast(F32),
                                ident)
                        copy_out(dstT[:, :, ts(sc, P)],
                                 tp.rearrange("p (c q) -> p c q", c=DC))

                    def emit_vn(jc):
                        ps = pjps.tile([P, U], F32, tag="pj")
                        for dc in range(DC):
                            nc.tensor.matmul(
                                ps, vT[:, dc, ts(jc, P)], w3[:, dc, :],
                                start=(dc == 0), stop=(dc == DC - 1))
                        copy_out(vN[:, jc, :], ps)

                    def emit_kt(ib):
                        for uc in range(UC):
                            ps = pjps.tile([P, IB], F32, tag="pj")
                            for dc in range(DC):
                                nc.tensor.matmul(
                                    ps, w2[:, dc, ts(uc, P)],
                                    vT[:, dc, ts(ib, IB)],
                                    start=(dc == 0), stop=(dc == DC - 1))
                            copy_out(kT[:, uc, ts(ib, IB)], ps)

                    def emit_qt(ib):
                        for uc in range(UC):
                            ps = pjps.tile([P, IB], F32, tag="pj")
                            for dc in range(DC):
                                nc.tensor.matmul(
                                    ps, w1[:, dc, ts(uc, P)],
                                    xT[:, dc, ts(ib, IB)],
                                    start=(dc == 0), stop=(dc == DC - 1))
                            copy_out(qT[:, uc, ts(ib, IB)], ps)

                    # Interleave DMA arrival with PE work. Projections run one
                    # chunk behind the transposes so the PSUM->SBUF copy of
                    # chunk jc completes while PE transposes chunk jc+1.
                    nc.sync.dma_start(w3, w3_d.rearrange("(c p) u -> p c u", p=P))
                    for jc in range(SC):
                        if jc % 4 == 0:
                            nat4 = loadp.tile([P, 4, D], F32R, tag="nat",
                                              name=f"nat_v{jc // 4}")
                            nc.sync.dma_start(
                                nat4, v_d[ts(jc // 4, 4 * P), :].rearrange(
                                    "(c p) d -> p c d", p=P))
                        if jc == 1:
                            nc.sync.dma_start(
                                w2, w2_d.rearrange("(c p) u -> p c u", p=P))
                        transpose_chunk(nat4[:, jc % 4, :], vT, jc)
                        if jc > 0:
                            emit_vn(jc - 1)
                        if jc % 4 == 0 and jc > 0:
                            emit_kt(jc // 4 - 1)
                    emit_vn(SC - 1)
                    # X side: transpose each chunk; qT one i-block behind
                    for sc in range(SC):
                        if sc % 4 == 0:
                            nat4 = loadp.tile([P, 4, D], F32R, tag="nat",
                                              name=f"nat_x{sc // 4}")
                            nc.sync.dma_start(
                                nat4, q_d[ts(sc // 4, 4 * P), :].rearrange(
                                    "(c p) d -> p c d", p=P))
                        if sc == 1:
                            nc.sync.dma_start(
                                w1, w1_d.rearrange("(c p) u -> p c u", p=P))
                        transpose_chunk(nat4[:, sc % 4, :], xT, sc)
                        if sc == 0:
                            emit_kt(NIB - 1)
                        if sc % 4 == 0 and sc > 0:
                            emit_qt(sc // 4 - 1)
                    emit_qt(NIB - 1)

                # ---- phase 2: attention ----
                with tc.tile_pool(name="expp", bufs=2) as expp, \
                     tc.tile_pool(name="scps", bufs=2, space="PSUM") as scps, \
                     tc.tile_pool(name="ctps", bufs=2, space="PSUM") as ctps, \
                     tc.tile_pool(name="dnps", bufs=2, space="PSUM") as dnps, \
                     tc.tile_pool(name="tdps", bufs=2, space="PSUM") as tdps, \
                     tc.tile_pool(name="outp", bufs=3) as outp:
                    for ib in range(NIB):
                        expB = expp.tile([P, SC, IB], F32R, name="expB")
                        # denT[0, i] accumulates sum_j expS[j, i] for this
                        # i-block (ones is the 2-col stationary; row 1 unused)
                        denT = dnps.tile([2, IB], F32, tag="dn")
                        for jc in range(SC):
                            ps = scps.tile([P, IB], F32, tag="sc")
                            for uc in range(UC):
                                nc.tensor.matmul(
                                    ps, kT[:, uc, ts(jc, P)], qT[:, uc, ts(ib, IB)],
                                    start=(uc == 0), stop=(uc == UC - 1))
                            nc.scalar.activation(expB[:, jc, :], ps, EXP, scale=SCALE)
                        for jc in range(SC):
                            nc.tensor.matmul(
                                denT, ones, expB[:, jc, :],
                                start=(jc == 0), stop=(jc == SC - 1))
                        # denominator row -> per-partition column via PE
                        # transpose of 128-wide slices
                        denTs = outp.tile([1, IB], F32, tag="denTs")
                        nc.vector.tensor_copy(denTs, denT[0:1, :])
                        dcol = tdps.tile([P, ICC], F32, tag="dcol")
                        for icc in range(ICC):
                            nc.tensor.transpose(
                                dcol[:, icc:icc + 1], denTs[0:1, ts(icc, P)],
                                ident[0:1, 0:1])
                        for icc in range(ICC):
                            i_glob = ib * ICC + icc
                            recip = outp.tile([P, 1], F32, tag="recip")
                            nc.vector.reciprocal(recip, dcol[:, icc:icc + 1])
                            cps = ctps.tile([P, U], F32, tag="ct")
                            for jc in range(SC):
                                nc.tensor.matmul(
                                    cps, expB[:, jc, ts(icc, P)], vN[:, jc, :],
                                    start=(jc == 0), stop=(jc == SC - 1))
                            co = outp.tile([P, U], F32, tag="co")
                            nc.vector.tensor_scalar_mul(co, cps, recip)
                            nc.sync.dma_start(o_d[ts(i_glob, P), :], co)


_PROGRAM = None


def _get_program():
    global _PROGRAM
    if _PROGRAM is None:
        nc = bacc.Bacc("TRN2", target_bir_lowering=False, debug=False,
                       num_devices=B)
        q_d = nc.dram_tensor("query", (S, D), F32R, kind="ExternalInput").ap()
        v_d = nc.dram_tensor("value", (S, D), F32R, kind="ExternalInput").ap()
        w1_d = nc.dram_tensor("W1", (D, U), F32R, kind="ExternalInput").ap()
        w2_d = nc.dram_tensor("W2", (D, U), F32R, kind="ExternalInput").ap()
        w3_d = nc.dram_tensor("W3", (D, U), F32R, kind="ExternalInput").ap()
        o_d = nc.dram_tensor("out", (S, U), F32, kind="ExternalOutput").ap()
        with tile.TileContext(nc) as tc:
            _emit(nc, tc, q_d, v_d, w1_d, w2_d, w3_d, o_d)
        nc.compile()
        _PROGRAM = nc
    return _PROGRAM


def kernel(**inputs) -> np.ndarray:
    query = np.ascontiguousarray(inputs["query"], dtype=np.float32)
    value = np.ascontiguousarray(inputs["value"], dtype=np.float32)
    W1 = np.ascontiguousarray(inputs["W1"], dtype=np.float32)
    W2 = np.ascontiguousarray(inputs["W2"], dtype=np.float32)
    W3 = np.ascontiguousarray(inputs["W3"], dtype=np.float32)
    assert query.shape == (B, S, D) and value.shape == (B, S, D)

    nc = _get_program()
    in_maps = [
        {"query": query[b], "value": value[b], "W1": W1, "W2": W2, "W3": W3}
        for b in range(B)
    ]
    res = run_bass_kernel_spmd(nc, in_maps, core_ids=list(range(B)))
    return np.stack([res.results[b]["out"] for b in range(B)], axis=0)
